# revision 1
# baseline (speedup 1.0000x reference)
"""Low_Rank_linear Trainium2 kernel.

Math (reference):
    hidden = (x[..., col_idx] * wnorm) @ B.T            # [tok, 512]
    y[..., row_idx]      = hidden @ A.T + x[..., col_comp_idx] @ sparse1.T
    y[..., row_comp_idx] = x @ sparse2.T

Reformulation used here (all index handling folded into host-built weights):
    u = x @ W1.T        W1 = [Bs; G; sparse2]  (1024 x 4096)
        Bs[:, col_idx]        = B * wnorm      (rank rows scattered to full width)
        G[i, col_comp_idx[i]] = 1              (one-hot gather of comp columns)
    y = u @ W2.T        W2 (4096 x 1024), rows interleaved on host:
        W2[row_idx[j]]      = [A[j] | sparse1[j] | 0]
        W2[row_comp_idx[i]] = [0    | 0          | e_i]
    so y comes out of the second matmul already in natural column order.

Sharding: data-parallel over the 8192 tokens -> 1024 tokens per core, weights
replicated. No collectives. Matmuls run in bf16 with fp32 PSUM accumulation.
"""

import numpy as np
import ml_dtypes

import concourse.bacc as bacc
import concourse.tile as tile
import concourse.mybir as mybir
from concourse.bass_utils import run_bass_kernel_spmd

N_CORES = 8
TOK = 8192            # 4 * 2048 tokens total
TPC = TOK // N_CORES  # 1024 tokens per core
N = 4096              # model width (in == out)
RANK = 512
NCOMP = 256           # complement set size (both col and row)
KU = RANK + NCOMP + NCOMP  # 1024 = width of intermediate u
BLK = 512             # token block (matmul moving N)
TT = 128              # token tile (partition dim)

_BF16 = mybir.dt.bfloat16
_F32 = mybir.dt.float32


def _build_nc():
    nc = bacc.Bacc(None)
    x_d = nc.dram_tensor("x", [TPC, N], _F32, kind="ExternalInput")
    w1_d = nc.dram_tensor("w1t", [N, KU], _BF16, kind="ExternalInput")
    w2_d = nc.dram_tensor("w2t", [KU, N], _BF16, kind="ExternalInput")
    y_d = nc.dram_tensor("y", [TPC, N], _F32, kind="ExternalOutput")

    n_blk = TPC // BLK          # 2 token blocks per core
    tpb = BLK // TT             # 4 token tiles per block
    k1 = N // 128               # 32 k-tiles for matmul A
    m1 = KU // 128              # 8 u-feature tiles
    k2 = KU // 128              # 8 k-tiles for matmul B
    n2 = N // BLK               # 8 output-feature chunks

    with tile.TileContext(nc) as tc:
        with (
            tc.tile_pool(name="w1", bufs=1) as w1_pool,
            tc.tile_pool(name="w2", bufs=2) as w2_pool,
            tc.tile_pool(name="xb", bufs=2) as xb_pool,
            tc.tile_pool(name="xt", bufs=2) as xt_pool,
            tc.tile_pool(name="u3", bufs=2) as u3_pool,
            tc.tile_pool(name="yo", bufs=4) as yo_pool,
            tc.tile_pool(name="psA", bufs=2, space="PSUM") as psA,
            tc.tile_pool(name="psB", bufs=2, space="PSUM") as psB,
        ):
            # resident W1.T in SBUF: [128, 32 k-tiles, 1024]
            w1_sb = w1_pool.tile([128, k1, KU], _BF16)
            nc.sync.dma_start(
                w1_sb[:], w1_d.rearrange("(kt p) m -> p kt m", p=128)
            )

            for blk in range(n_blk):
                t0 = blk * BLK
                # load + cast x to bf16 (token-major), then DMA-transpose to
                # feature-major xt [128 feat, k-tile, 512 tok]
                xt_sb = xt_pool.tile([128, k1, BLK], _BF16)
                for tt in range(tpb):
                    xb = xb_pool.tile([128, N], _BF16)
                    nc.gpsimd.dma_start(
                        xb[:], x_d[t0 + tt * TT : t0 + (tt + 1) * TT, :]
                    )
                    nc.sync.dma_start_transpose(
                        xt_sb[:, :, tt * TT : (tt + 1) * TT], xb[:]
                    )

                # MM-A: u.T [ufeat, tok] = W1 @ x.T ; cast to bf16
                u3_sb = u3_pool.tile([128, k2, BLK], _BF16)
                for m in range(m1):
                    ps = psA.tile([128, BLK], _F32)
                    for kt in range(k1):
                        nc.tensor.matmul(
                            ps[:],
                            w1_sb[:, kt, m * 128 : (m + 1) * 128],
                            xt_sb[:, kt, :],
                            start=(kt == 0),
                            stop=(kt == k1 - 1),
                        )
                    nc.vector.tensor_copy(out=u3_sb[:, m, :], in_=ps[:])

                # MM-B: y [tok, outfeat] = u @ W2.T, n-chunk at a time
                for n in range(n2):
                    w2_sb = w2_pool.tile([128, k2, BLK], _BF16)
                    nc.sync.dma_start(
                        w2_sb[:],
                        w2_d.rearrange("(kt p) n -> p kt n", p=128)[
                            :, :, n * BLK : (n + 1) * BLK
                        ],
                    )
                    for mt in range(tpb):
                        ps = psB.tile([128, BLK], _F32)
                        for kt in range(k2):
                            nc.tensor.matmul(
                                ps[:],
                                u3_sb[:, kt, mt * TT : (mt + 1) * TT],
                                w2_sb[:, kt, :],
                                start=(kt == 0),
                                stop=(kt == k2 - 1),
                            )
                        yo = yo_pool.tile([128, BLK], _F32)
                        nc.vector.tensor_copy(out=yo[:], in_=ps[:])
                        nc.sync.dma_start(
                            y_d[
                                t0 + mt * TT : t0 + (mt + 1) * TT,
                                n * BLK : (n + 1) * BLK,
                            ],
                            yo[:],
                        )
    nc.finalize()
    return nc


_NC_CACHE = {}


def get_nc():
    if "nc" not in _NC_CACHE:
        _NC_CACHE["nc"] = _build_nc()
    return _NC_CACHE["nc"]


def _prep_weights(A, B, sparse_weights1, sparse_weights2, weights_norms_rowwise,
                  col_idx, col_comp_idx, row_idx, row_comp_idx):
    bf16 = ml_dtypes.bfloat16
    # W1 = [Bs; G; sparse2]  (1024, 4096)
    w1 = np.zeros((KU, N), dtype=np.float32)
    w1[:RANK, col_idx] = B * weights_norms_rowwise[None, :]
    w1[RANK + np.arange(NCOMP), col_comp_idx] = 1.0
    w1[RANK + NCOMP :, :] = sparse_weights2
    # W2 (4096, 1024) with interleaved rows; build transposed directly
    w2t = np.zeros((KU, N), dtype=np.float32)
    w2t[:RANK, row_idx] = A.T
    w2t[RANK : RANK + NCOMP, row_idx] = sparse_weights1.T
    w2t[RANK + NCOMP + np.arange(NCOMP), row_comp_idx] = 1.0
    w1t = np.ascontiguousarray(w1.T).astype(bf16)       # [4096, 1024]
    w2t = np.ascontiguousarray(w2t).astype(bf16)        # [1024, 4096]
    return w1t, w2t


def kernel(x, A, B, sparse_weights1, sparse_weights2, weights_norms_rowwise,
           col_idx, col_comp_idx, row_idx, row_comp_idx):
    x = np.asarray(x, dtype=np.float32)
    w1t, w2t = _prep_weights(
        np.asarray(A, np.float32), np.asarray(B, np.float32),
        np.asarray(sparse_weights1, np.float32),
        np.asarray(sparse_weights2, np.float32),
        np.asarray(weights_norms_rowwise, np.float32),
        np.asarray(col_idx), np.asarray(col_comp_idx),
        np.asarray(row_idx), np.asarray(row_comp_idx),
    )
    nc = get_nc()
    xs = np.ascontiguousarray(x.reshape(TOK, N))
    in_maps = [
        {"x": xs[c * TPC : (c + 1) * TPC], "w1t": w1t, "w2t": w2t}
        for c in range(N_CORES)
    ]
    res = run_bass_kernel_spmd(nc, in_maps, core_ids=list(range(N_CORES)))
    globals()["_LAST_RESULTS"] = res
    y = np.concatenate([res.results[c]["y"] for c in range(N_CORES)], axis=0)
    return np.ascontiguousarray(y.reshape(x.shape).astype(np.float32))



# revision 2
# speedup vs baseline: 1.0752x; 1.0752x over previous
"""Low_Rank_linear Trainium2 kernel, v5.

Per 512-token block (data-parallel over 8 cores, host-permuted inputs,
x pre-transposed feature-major bf16 -- see v2-v4 history in git... err,
in the transcript):
    MM-A  hidden.T = (B*wnorm) @ xp.T          k=3840, out 512  bf16
    MM-B  yp[:,:3840] = hid @ A.T + xc @ s1.T  k=768, out 3840  bf16+fp8DR
    MM-C  y2 = (s2p*64) @ xp.T (feature-major) k=4096, out 256  fp8DR

v5 vs v4 (171.2us): startup is *bytes*-bound (~250GB/s aggregate DMA);
B0 cannot start before ~45us because it needs w1+x(blk0)+w2 ~= 13MB.
v4 paid that wait as a 10us PE gap that also re-throttled the HAM
clock to 1.2GHz for B0's first 14us.  Changes:
  - block 0 runs A->C->B: MM-C (needs only s2, 1MB, loaded early + the
    on-device fp8 cast of x) bridges the w2-arrival wait, so the PE
    never idles >3.4us and B0 starts warm.  Block 1 runs A->B->C so the
    tiny y2 write, not the last 1MB y row write, ends the kernel.
  - first w1/x chunks halved to 4 k-tiles: first matmul ~4us earlier
  - 26 warm-up matmuls (48 in v4 overshot and delayed A0)
  - y row-tile written as two 480KB halves, first issued mid-row
"""

import numpy as np
import ml_dtypes

import concourse.bacc as bacc
import concourse.tile as tile
import concourse.mybir as mybir
from concourse.bass_utils import run_bass_kernel_spmd

N_CORES = 8
TOK = 8192
TPC = TOK // N_CORES  # 1024 tokens per core
N = 4096
RANK = 512
NKEEP = 3840
NCOMP = 256
BLK = 512             # token block (matmul moving N)
TT = 128              # token tile (stationary partition dim)
NBLK = TPC // BLK     # 2
KT_ALL = N // 128     # 32
KT_A = NKEEP // 128   # 30
KT_B = RANK // 128    # 4
NCH = 8
CW = NKEEP // NCH     # 480
XCK = 8               # k-tiles per full x chunk
S1S = 8.0
S2S = 64.0
NDUMMY = 26
HW = NKEEP // 2       # 1920, y half-row width

_BF16 = mybir.dt.bfloat16
_F32 = mybir.dt.float32
_F8 = mybir.dt.float8e4
_DR = mybir.MatmulPerfMode.DoubleRow


def _build_nc():
    nc = bacc.Bacc(None)
    x_d = nc.dram_tensor("x", [NBLK, 4, 128, XCK, BLK], _BF16, kind="ExternalInput")
    w1_d = nc.dram_tensor("w1", [2, 128, 15, RANK], _BF16, kind="ExternalInput")
    w2_d = nc.dram_tensor("w2", [128, KT_B, NKEEP], _BF16, kind="ExternalInput")
    s1_d = nc.dram_tensor("s1", [128, 2, NKEEP], _F8, kind="ExternalInput")
    s2_d = nc.dram_tensor("s2", [128, KT_ALL, NCOMP], _F8, kind="ExternalInput")
    y_d = nc.dram_tensor("y", [TPC, NKEEP], _BF16, kind="ExternalOutput")
    y2_d = nc.dram_tensor("y2", [NCOMP, TPC], _BF16, kind="ExternalOutput")

    with tile.TileContext(nc) as tc:
        with (
            tc.tile_pool(name="w14", bufs=2) as w14_pool,
            tc.tile_pool(name="w1", bufs=3) as w1_pool,
            tc.tile_pool(name="w2", bufs=4) as w2_pool,
            tc.tile_pool(name="s1", bufs=1) as s1_pool,
            tc.tile_pool(name="s2", bufs=1) as s2_pool,
            tc.tile_pool(name="xt4", bufs=2) as xt4_pool,
            tc.tile_pool(name="xt", bufs=7) as xt_pool,
            tc.tile_pool(name="x8", bufs=1) as x8_pool,
            tc.tile_pool(name="xc8", bufs=2) as xc8_pool,
            tc.tile_pool(name="u3", bufs=2) as u3_pool,
            tc.tile_pool(name="yoa", bufs=2) as yoa_pool,
            tc.tile_pool(name="yob", bufs=2) as yob_pool,
            tc.tile_pool(name="yc", bufs=2) as yc_pool,
            tc.tile_pool(name="wrm", bufs=1) as wrm_pool,
            tc.tile_pool(name="psA", bufs=4, space="PSUM") as psA,
            tc.tile_pool(name="psB", bufs=2, space="PSUM") as psB,
            tc.tile_pool(name="psC", bufs=2, space="PSUM") as psC,
        ):
            # --- tiles ---------------------------------------------------
            # blk0 chunk layout: two 4kt halves then three 8kt chunks
            w1h = [w14_pool.tile([128, 4, RANK], _BF16, name="w14t")
                   for _ in range(2)]
            w1f = [w1_pool.tile([128, XCK, RANK], _BF16, name="w1sb")
                   for _ in range(3)]
            # (tile, kt_start, nkt, m-slices source) per A-chunk, shared idx
            w1_chunks = [(w1h[0], 0, 4), (w1h[1], 4, 4), (w1f[0], 8, 8),
                         (w1f[1], 16, 8), (w1f[2], 24, 6)]
            w2_sb = [w2_pool.tile([128, KT_B, 2 * CW], _BF16, name="w2sb")
                     for c in range(4)]
            s1_sb = s1_pool.tile([128, 2, NKEEP], _F8)
            s2_sb = s2_pool.tile([128, KT_ALL, NCOMP], _F8)
            xt0h = [xt4_pool.tile([128, 4, BLK], _BF16, name="xt4t")
                    for _ in range(2)]
            xt_f = [xt_pool.tile([128, XCK, BLK], _BF16, name="xts")
                    for _ in range(7)]
            # per-block x chunk lists: (tile, kt_start, nkt)
            x_chunks = [
                [(xt0h[0], 0, 4), (xt0h[1], 4, 4), (xt_f[0], 8, 8),
                 (xt_f[1], 16, 8), (xt_f[2], 24, 8)],
                [(xt_f[3], 0, 8), (xt_f[4], 8, 8), (xt_f[5], 16, 8),
                 (xt_f[6], 24, 8)],
            ]
            x8_sb = x8_pool.tile([128, KT_ALL, BLK], _F8)

            # --- warm-up (HAM 8/8 before real MMs) -----------------------
            wrm = wrm_pool.tile([128, 128], _BF16)
            wps = psA.tile([128, 128], _F32, name="psa")
            nc.gpsimd.memset(wrm[:], 0.0)
            for i in range(NDUMMY):
                nc.tensor.matmul(wps[:], wrm[:], wrm[:], start=True, stop=True)

            # --- need-ordered loads --------------------------------------
            nc.sync.dma_start(w1h[0][:], w1_d[0, :, :4])
            nc.scalar.dma_start(xt0h[0][:], x_d[0, 0, :, :4, :])
            nc.sync.dma_start(w1h[1][:], w1_d[0, :, 4:8])
            nc.scalar.dma_start(xt0h[1][:], x_d[0, 0, :, 4:, :])
            nc.sync.dma_start(w1f[0][:, :7, :], w1_d[0, :, 8:])
            nc.scalar.dma_start(xt_f[0][:], x_d[0, 1])
            nc.sync.dma_start(w1f[0][:, 7:8, :], w1_d[1, :, :1])
            nc.sync.dma_start(w1f[1][:], w1_d[1, :, 1:9])
            nc.scalar.dma_start(xt_f[1][:], x_d[0, 2])
            nc.sync.dma_start(w1f[2][:, :6, :], w1_d[1, :, 9:])
            nc.scalar.dma_start(xt_f[2][:], x_d[0, 3])
            nc.sync.dma_start(s2_sb[:], s2_d[:])
            nc.sync.dma_start(s1_sb[:], s1_d[:])
            for c in range(4):
                nc.sync.dma_start(w2_sb[c][:],
                                  w2_d[:, :, c * 2 * CW:(c + 1) * 2 * CW])
            nc.sync.dma_start(xt_f[3][:], x_d[1, 0])
            nc.scalar.dma_start(xt_f[4][:], x_d[1, 1])
            nc.sync.dma_start(xt_f[5][:], x_d[1, 2])
            nc.scalar.dma_start(xt_f[6][:], x_d[1, 3])

            # --- per-block compute ---------------------------------------
            def mm_a(blk):
                u3 = u3_pool.tile([128, KT_B, BLK], _BF16)
                psa = [psA.tile([128, BLK], _F32, name="psa")
                       for m in range(RANK // 128)]
                for (xt, kt0, nkt) in x_chunks[blk]:
                    for j in range(nkt):
                        kt = kt0 + j
                        if kt >= KT_A:
                            continue
                        ci = 0 if kt < 4 else 1 if kt < 8 else 2 + (kt - 8) // 8
                        wt, wkt0, _ = w1_chunks[ci]
                        for m in range(RANK // 128):
                            nc.tensor.matmul(
                                psa[m][:],
                                wt[:, kt - wkt0, m * 128:(m + 1) * 128],
                                xt[:, j, :],
                                start=(kt == 0),
                                stop=(kt == KT_A - 1),
                            )
                    # fp8 cast for MM-C, hidden in MM-A's DMA-paced window
                    nc.scalar.copy(out=x8_sb[:, kt0:kt0 + nkt, :], in_=xt[:])
                for m in range(RANK // 128):
                    nc.vector.tensor_copy(out=u3[:, m, :], in_=psa[m][:])
                xc8 = xc8_pool.tile([128, 2, BLK], _F8)
                nc.scalar.mul(xc8[:], x8_sb[:, 30:32, :], 1.0 / S1S)
                return u3, xc8

            def mm_b(blk, u3, xc8):
                t0 = blk * BLK
                for mt in range(BLK // TT):
                    yoa = yoa_pool.tile([128, HW], _BF16)
                    yob = yob_pool.tile([128, HW], _BF16)
                    for n in range(NCH):
                        ps = psB.tile([128, CW], _F32)
                        for kt in range(KT_B):
                            nc.tensor.matmul(
                                ps[:],
                                u3[:, kt, mt * TT:(mt + 1) * TT],
                                w2_sb[n // 2][:, kt,
                                              (n % 2) * CW:(n % 2 + 1) * CW],
                                start=(kt == 0),
                                stop=False,
                            )
                        nc.tensor.matmul(
                            ps[:],
                            xc8[:, :, mt * TT:(mt + 1) * TT],
                            s1_sb[:, :, n * CW:(n + 1) * CW],
                            start=False,
                            stop=True,
                            perf_mode=_DR,
                        )
                        dst = yoa if n < 4 else yob
                        nc.vector.tensor_copy(
                            out=dst[:, (n % 4) * CW:(n % 4 + 1) * CW], in_=ps[:]
                        )
                        if n == 3:
                            nc.scalar.dma_start(
                                y_d[t0 + mt * TT:t0 + (mt + 1) * TT, :HW],
                                yoa[:],
                            )
                    nc.scalar.dma_start(
                        y_d[t0 + mt * TT:t0 + (mt + 1) * TT, HW:], yob[:]
                    )

            def mm_c(blk):
                t0 = blk * BLK
                for mh in range(NCOMP // 128):
                    ps = psC.tile([128, BLK], _F32)
                    for k2 in range(KT_ALL // 2):
                        nc.tensor.matmul(
                            ps[:],
                            s2_sb[:, 2 * k2:2 * k2 + 2, mh * 128:(mh + 1) * 128],
                            x8_sb[:, 2 * k2:2 * k2 + 2, :],
                            start=(k2 == 0),
                            stop=(k2 == KT_ALL // 2 - 1),
                            perf_mode=_DR,
                        )
                    yc = yc_pool.tile([128, BLK], _BF16)
                    nc.scalar.mul(yc[:], ps[:], 1.0 / S2S)
                    nc.gpsimd.dma_start(
                        y2_d[mh * 128:(mh + 1) * 128, t0:t0 + BLK], yc[:]
                    )

            # blk0: A,C,B -- C bridges the w2-arrival wait, keeps HAM warm.
            # blk1: A,B,C -- the tiny y2 write ends the kernel, not a 480KB
            # y half-row.
            u3, xc8 = mm_a(0)
            mm_c(0)
            mm_b(0, u3, xc8)
            u3, xc8 = mm_a(1)
            mm_b(1, u3, xc8)
            mm_c(1)
    nc.finalize()
    return nc


_NC_CACHE = {}


def get_nc():
    if "nc" not in _NC_CACHE:
        _NC_CACHE["nc"] = _build_nc()
    return _NC_CACHE["nc"]


def _prep(A, B, sparse_weights1, sparse_weights2, weights_norms_rowwise,
          col_idx, col_comp_idx):
    bf16 = ml_dtypes.bfloat16
    f8 = ml_dtypes.float8_e4m3
    perm_in = np.concatenate([col_idx, col_comp_idx])
    w1t = (B * weights_norms_rowwise[None, :]).T.astype(np.float32)
    w1 = np.ascontiguousarray(
        w1t.reshape(2, 15, 128, RANK).transpose(0, 2, 1, 3)
    ).astype(bf16)
    w2 = np.ascontiguousarray(
        A.T.astype(np.float32).reshape(KT_B, 128, NKEEP).transpose(1, 0, 2)
    ).astype(bf16)
    s1 = np.ascontiguousarray(
        (sparse_weights1.T * S1S).astype(np.float32)
        .reshape(2, 128, NKEEP).transpose(1, 0, 2)
    ).astype(f8)
    s2t = (sparse_weights2[:, perm_in].T * S2S).astype(np.float32)
    s2 = np.ascontiguousarray(
        s2t.reshape(KT_ALL, 128, NCOMP).transpose(1, 0, 2)
    ).astype(f8)
    return w1, w2, s1, s2, perm_in


def kernel(x, A, B, sparse_weights1, sparse_weights2, weights_norms_rowwise,
           col_idx, col_comp_idx, row_idx, row_comp_idx):
    bf16 = ml_dtypes.bfloat16
    x = np.asarray(x, dtype=np.float32)
    w1, w2, s1, s2, perm_in = _prep(
        np.asarray(A, np.float32), np.asarray(B, np.float32),
        np.asarray(sparse_weights1, np.float32),
        np.asarray(sparse_weights2, np.float32),
        np.asarray(weights_norms_rowwise, np.float32),
        np.asarray(col_idx), np.asarray(col_comp_idx),
    )
    row_idx = np.asarray(row_idx)
    row_comp_idx = np.asarray(row_comp_idx)

    xs = x.reshape(TOK, N)
    in_maps = []
    for c in range(N_CORES):
        xcT = xs[c * TPC:(c + 1) * TPC][:, perm_in].T
        xb = np.ascontiguousarray(
            xcT.reshape(4, XCK, 128, NBLK, BLK).transpose(3, 0, 2, 1, 4)
        ).astype(bf16)
        in_maps.append({"x": xb, "w1": w1, "w2": w2, "s1": s1, "s2": s2})

    nc = get_nc()
    res = run_bass_kernel_spmd(nc, in_maps, core_ids=list(range(N_CORES)))
    globals()["_LAST_RESULTS"] = res
    y_rows = np.concatenate(
        [np.asarray(res.results[c]["y"], dtype=np.float32) for c in range(N_CORES)],
        axis=0,
    )
    y_comp = np.concatenate(
        [np.asarray(res.results[c]["y2"], dtype=np.float32) for c in range(N_CORES)],
        axis=1,
    )
    y = np.empty((TOK, N), dtype=np.float32)
    y[:, row_idx] = y_rows
    y[:, row_comp_idx] = y_comp.T
    return np.ascontiguousarray(y.reshape(x.shape))
